# revision 32
# baseline (speedup 1.0000x reference)
"""Trainium2 Bass kernel for nn_DependencyLinearLayer.

Math (collapsed-H reformulation of the reference):
  out[b,i,c,j] = dep_logits[dg[b,i,j], c] + s_log[b,i,c] + t_log[b,j,c] + cls_b[c]
where
  dep_logits = dep_emb @ w_d.T                  [48, 12]
  s_log      = x @ (w_s @ s_fc_w).T + w_s@s_fc_b  (combined-weight form)
  t_log      = x @ (w_t @ t_fc_w).T + w_t@t_fc_b
  w_s, w_t, w_d = cls_w[:, :H], cls_w[:, H:2H], cls_w[:, 2H:]

Sharding: 8 cores; core n handles batch b = n//2 and i-rows [128*(n%2), 128*(n%2)+128).

The per-element 48-entry table lookup runs on GPSIMD via ap_gather with a
PAIRED table: table2[48*a+b] = (T[a], T[b]) so each gather index produces two
consecutive output j's, halving the Q7 read-command count (the bottleneck).
All floating-point math (projections, table construction, broadcast adds)
runs on-device (PE/ACT/DVE); the host only reshapes/shards inputs.
"""

import os
import sys

import numpy as np

for _p in ("/opt/trn_rl_repo",):
    if _p not in sys.path:
        sys.path.insert(0, _p)

import concourse.bass as bass  # noqa: E402
import concourse.tile as tile  # noqa: E402
from concourse import bacc, mybir  # noqa: E402
from concourse.tile import ScopedClock, add_dep_helper  # noqa: E402

B, L, IN, H, C, NDEP = 4, 256, 768, 256, 12, 48
NCORES = 8
RPC = L // 2  # i-rows per core (128)
NINST = 8    # ap_gather instructions per core (2 i-rows x 8 groups each)

_MAX_TAIL_WAITS = 2


def _patched_drain_and_barrier(self, tick_clock, wait_clock):
    # The walrus build in this image rejects >1 sync-wait on one CTRL
    # instruction; split the kernel-tail drain waits across nops.
    drain_inst = self.nc.sync.drain()
    wait_clock.add_sem_waits(
        drain_inst.ins, ScopedClock({None: tick_clock.global_clock})
    )
    sync_info = drain_inst.ins.sync_info
    if sync_info is not None and len(sync_info.on_wait) > _MAX_TAIL_WAITS:
        waits = list(sync_info.on_wait)
        sync_info.on_wait = waits[:_MAX_TAIL_WAITS]
        rest = waits[_MAX_TAIL_WAITS:]
        while rest:
            chunk, rest = rest[:_MAX_TAIL_WAITS], rest[_MAX_TAIL_WAITS:]
            nop = self.nc.sync.nop(nofuse=True, hint="tail_drain_split").ins
            nop.sync_info = mybir.SyncInfo(on_wait=chunk, on_update=[])
    self.nc.all_engine_barrier()
    assert self.sems is not None
    popped = self.nc._tile_sem_poison_stack.pop()
    assert popped is self._sem_poison
    self.nc.clear_and_free_semaphores(list(self.sems.allocated().values()))
    self.nc.all_engine_barrier()


tile.TileContext._drain_and_barrier = _patched_drain_and_barrier

_PROGRAM = None

# raw block order (block=2u+h, g) -> i_loc = 16u + 8h + g; _ROWPERM[i_loc] = block*8+g
_ROWPERM = np.zeros(RPC, dtype=np.int64)
for _u in range(NINST):
    for _h in range(2):
        for _g in range(8):
            _ROWPERM[16 * _u + 8 * _h + _g] = (2 * _u + _h) * 8 + _g


def build_program():
    f32 = mybir.dt.float32
    nc = bacc.Bacc("TRN2", target_bir_lowering=False, debug=False)

    xbT = nc.declare_dram_parameter("xbT", [IN, L], f32, isOutput=False)
    xsT = nc.declare_dram_parameter("xsT", [IN, RPC], f32, isOutput=False)
    dgw = nc.declare_dram_parameter("dgw", [128, 128], mybir.dt.int16, isOutput=False)
    sfw = nc.declare_dram_parameter("sfw", [H, IN], f32, isOutput=False)
    tfw = nc.declare_dram_parameter("tfw", [H, IN], f32, isOutput=False)
    # packed small weights: cols [wsT|wtT|wdT|depT|sfb|tfb] = 12+12+12+48+1+1
    wpack = nc.declare_dram_parameter("wpack", [H, 86], f32, isOutput=False)
    clbT = nc.declare_dram_parameter("clbT", [C, 1], f32, isOutput=False)
    repmat = nc.declare_dram_parameter("repmat", [C, 128], f32, isOutput=False)
    out_d = nc.declare_dram_parameter("out", [RPC * 16, L], f32, isOutput=True)

    Copy = mybir.ActivationFunctionType.Copy

    with tile.TileContext(nc) as tc:
        with (
            tc.tile_pool(name="const", bufs=1) as cp,
            tc.tile_pool(name="gpool", bufs=8) as gp,
            tc.tile_pool(name="psum", bufs=1, space="PSUM") as pp,
            tc.tile_pool(name="psum2", bufs=1, space="PSUM") as pp2,
        ):
            # ---- input loads: small gather-critical first (sync queue
            # spreads across DMA engines; scalar queue serializes ~1.3us/desc) ----
            warm_tab = cp.tile([128, 4], f32, tag="warm_tab")
            warm_out = cp.tile([128, 16], f32, tag="warm_out")
            wpk = cp.tile([128, 172], f32, tag="wpk")
            nc.sync.dma_start(
                wpk[:].rearrange("p (h q) -> p h q", h=2),
                wpack[:].rearrange("(h p) q -> p h q", h=2),
            )

            def wp(h0, a, b):
                return wpk[:, h0 * 86 + a: h0 * 86 + b]
            clbT_t = cp.tile([C, 1], f32, tag="clbT_t")
            nc.sync.dma_start(clbT_t[:], clbT[:])
            rep_t = cp.tile([C, 128], f32, tag="rep_t")
            nc.sync.dma_start(rep_t[:], repmat[:])
            dgw_t = cp.tile([128, 128], mybir.dt.int16, tag="dgw_t")
            nc.sync.dma_start(dgw_t[:], dgw[:])
            with tc.high_priority():
                warm_src = wpk[:, 24:28]
                nc.vector.tensor_scalar_mul(warm_tab[:], warm_src, 0.0)
                nc.gpsimd.ap_gather(
                    warm_out[:], warm_tab[:],
                    warm_tab[:, 0:1].bitcast(mybir.dt.int16)[:, 0:1],
                    channels=128, num_elems=4, d=1, num_idxs=16,
                )
            # big loads after the critical batch
            sfw_t = cp.tile([128, 2 * IN], f32, tag="sfw_t")
            tfw_t = cp.tile([128, 2 * IN], f32, tag="tfw_t")
            for h0 in range(2):
                nc.sync.dma_start(sfw_t[:, h0 * IN:(h0 + 1) * IN], sfw[h0 * 128:(h0 + 1) * 128, :])
                nc.sync.dma_start(tfw_t[:, h0 * IN:(h0 + 1) * IN], tfw[h0 * 128:(h0 + 1) * 128, :])
            x_t = cp.tile([128, 6 * L], f32, tag="x_t")
            for m in range(6):
                nc.sync.dma_start(x_t[:, m * L:(m + 1) * L], xbT[m * 128:(m + 1) * 128, :])
            xs_t = cp.tile([128, 6 * RPC], f32, tag="xs_t")
            for m in range(6):
                nc.sync.dma_start(xs_t[:, m * RPC:(m + 1) * RPC], xsT[m * 128:(m + 1) * 128, :])

            # ---- dep_logitsT+biases [12, 48] in ONE accumulation group ----
            # pd[c,d] = sum_h wd[h,c]*depT[h,d] + ws@sfb + wt@tfb (bcast over d)
            with tc.high_priority():
                pd = pp.tile([C, NDEP], f32, tag="pd")
                nc.tensor.matmul(pd[:], wp(0, 24, 36), wp(0, 36, 84),
                                 start=True, stop=False)
                nc.tensor.matmul(pd[:], wp(1, 24, 36), wp(1, 36, 84),
                                 start=False, stop=False)
                nc.tensor.matmul(pd[:], wp(0, 0, 12),
                                 wp(0, 84, 85).broadcast_to([128, NDEP]),
                                 start=False, stop=False)
                nc.tensor.matmul(pd[:], wp(1, 0, 12),
                                 wp(1, 84, 85).broadcast_to([128, NDEP]),
                                 start=False, stop=False)
                nc.tensor.matmul(pd[:], wp(0, 12, 24),
                                 wp(0, 85, 86).broadcast_to([128, NDEP]),
                                 start=False, stop=False)
                pd_last = nc.tensor.matmul(pd[:], wp(1, 12, 24),
                                 wp(1, 85, 86).broadcast_to([128, NDEP]),
                                 start=False, stop=True)

            with tc.high_priority():
                midbufD = cp.tile([C, NDEP], f32, tag="midbufD")
                nc.scalar.activation(midbufD[:], pd[:],
                                     mybir.ActivationFunctionType.Identity, bias=clbT_t[:])
                prepD = pp2.tile([128, NDEP], f32, tag="prepD")
                prep_mm = nc.tensor.matmul(prepD[:], rep_t[:], midbufD[:],
                                           start=True, stop=True)
                t16_t = cp.tile([128, NDEP], f32, tag="t16_t")
                nc.vector.tensor_copy(t16_t[:], prepD[:])

                table2 = cp.tile([128, NDEP * NDEP * 2], f32, tag="table2")
                tv = table2[:].rearrange("p (a b t) -> p a b t", a=NDEP, b=NDEP, t=2)
                nc.vector.tensor_copy(
                    tv[:, :, :, 0], t16_t[:].unsqueeze(2).broadcast_to([128, NDEP, NDEP])
                )
                copy_a = nc.vector.tensor_copy(
                    tv[:, 0:12, :, 1],
                    t16_t[:].unsqueeze(1).broadcast_to([128, 12, NDEP]),
                )
                copy_b = nc.scalar.copy(
                    tv[:, 12:NDEP, :, 1],
                    t16_t[:].unsqueeze(1).broadcast_to([128, NDEP - 12, NDEP]),
                )

            # ---- combined weights W2[k, 0:12]=swT, [12:24]=twT ----
            w2_t = cp.tile([128, 6 * 24], f32, tag="w2_t")
            for m in range(6):
                pw = pp.tile([128, 24], f32, tag="pw")
                for h0 in range(2):
                    mm = nc.tensor.matmul(
                        pw[:, 0:C],
                        sfw_t[:, h0 * IN + m * 128: h0 * IN + (m + 1) * 128],
                        wp(h0, 0, 12),
                        start=(h0 == 0), stop=(h0 == 1),
                    )
                    if m == 0 and h0 == 0:
                        add_dep_helper(mm.ins, pd_last.ins, sync=False,
                                       reason="pd group first on PE")
                        add_dep_helper(mm.ins, prep_mm.ins, sync=False,
                                       reason="prepD before combine on PE")
                for h0 in range(2):
                    nc.tensor.matmul(
                        pw[:, C:2 * C],
                        tfw_t[:, h0 * IN + m * 128: h0 * IN + (m + 1) * 128],
                        wp(h0, 12, 24),
                        start=(h0 == 0), stop=(h0 == 1),
                    )
                w2c = nc.vector.tensor_copy(w2_t[:, m * 24:(m + 1) * 24], pw[:])
                if m == 0:
                    add_dep_helper(w2c.ins, copy_a.ins, sync=False,
                                   reason="table copyA first on DVE")


            # ---- projections: s_logT [12, 256] and t_logT [12, 256] ----
            ps = pp2.tile([C, RPC], f32, tag="ps")
            pt = pp2.tile([C, L], f32, tag="pt")
            for m in range(6):
                nc.tensor.matmul(
                    ps[:], w2_t[:, m * 24: m * 24 + C], xs_t[:, m * RPC:(m + 1) * RPC],
                    start=(m == 0), stop=(m == 5),
                )
            for m in range(6):
                nc.tensor.matmul(
                    pt[:], w2_t[:, m * 24 + C: m * 24 + 2 * C], x_t[:, m * L:(m + 1) * L],
                    start=(m == 0), stop=(m == 5),
                )



            # ---- t/s projections epilogue ----
            midbufT = cp.tile([C, L], f32, tag="midbufT")
            mbt = nc.scalar.activation(midbufT[:], pt[:], Copy)
            add_dep_helper(mbt.ins, copy_b.ins, sync=False,
                           reason="table copyB first on ACT")
            slog_t = cp.tile([C, RPC], f32, tag="slog_t")
            nc.scalar.activation(slog_t[:], ps[:], Copy)
            prepT = pp2.tile([128, L], f32, tag="prepT")
            nc.tensor.matmul(prepT[:], rep_t[:], midbufT[:], start=True, stop=True)
            t16u16 = cp.tile([128, L], f32, tag="t16u16")
            nc.vector.tensor_copy(t16u16[:], prepT[:])

            # ---- S_all [128, 16]: per-(instruction, half) per-partition scalars ----
            # S_all[16g+c, 2u+h] = s_logT[c, 16u+8h+g]
            s_all = cp.tile([128, 16], f32, tag="s_all")
            nc.vector.tensor_scalar_mul(s_all[:], sfw_t[:, 0:16], 0.0)
            for g in range(8):
                nc.scalar.dma_start(
                    s_all[16 * g:16 * g + C, :],
                    slog_t[0:C, g:g + 121:8],
                )

            # ---- gathers + fused adds + stores ----
            u16 = t16u16[:]
            g_tiles = []
            gather_insts = []
            for v in range(3):
                g_t = gp.tile([128, 4 * L], f32, tag="g_t")
                gi = nc.gpsimd.ap_gather(
                    g_t[:].rearrange("p (k t) -> p k t", t=2),
                    table2[:].rearrange("p (e t) -> p e t", t=2),
                    dgw_t[:, 32 * v:32 * (v + 1)],
                    channels=128, num_elems=NDEP * NDEP, d=2, num_idxs=512,
                )
                g_tiles.append(g_t)
                gather_insts.append(gi)
            # last block-pair split in two so its first half's epilogue
            # overlaps the second half's gather
            g_t3 = gp.tile([128, 4 * L], f32, tag="g_t3")
            for half in range(2):
                gi = nc.gpsimd.ap_gather(
                    g_t3[:, half * 2 * L:(half + 1) * 2 * L].rearrange(
                        "p (k t) -> p k t", t=2),
                    table2[:].rearrange("p (e t) -> p e t", t=2),
                    dgw_t[:, 96 + 16 * half:96 + 16 * (half + 1)],
                    channels=128, num_elems=NDEP * NDEP, d=2, num_idxs=256,
                )
                gather_insts.append(gi)
            g_tiles.append(g_t3)
            # adds + stores AFTER all gathers (DVE shares the GPSIMD SBUF port)
            last_g = gather_insts[3]  # first half of the split pair
            for v in range(4):
                g_t = g_tiles[v]
                for bidx in range(4):
                    aa = nc.vector.affine_then_add(
                        g_t[:, bidx * L:(bidx + 1) * L],
                        g_t[:, bidx * L:(bidx + 1) * L],
                        u16,
                        1.0,
                        s_all[:, 4 * v + bidx:4 * v + bidx + 1],
                    )
                    if v < 3:
                        add_dep_helper(aa.ins, last_g.ins, sync=False,
                                       reason="defer adds past gathers")
                if v < 3:
                    nc.sync.dma_start(
                        out_d[4 * v * 128:(4 * v + 4) * 128, :].rearrange(
                            "(b p) j -> p b j", b=4
                        ),
                        g_t[:].rearrange("p (b j) -> p b j", b=4),
                    )
                else:
                    for half in range(2):
                        nc.sync.dma_start(
                            out_d[(12 + 2 * half) * 128:(14 + 2 * half) * 128, :]
                            .rearrange("(b p) j -> p b j", b=2),
                            g_t[:, half * 2 * L:(half + 1) * 2 * L]
                            .rearrange("p (b j) -> p b j", b=2),
                        )

    nc.compile()
    return nc


def _marshal_core(n, input_tensor, dg, consts):
    b, half = n // 2, n % 2
    i0 = half * RPC
    dgb = dg[b]
    # paired indices, wrapped per 16-partition group:
    # instruction u, group g -> stream of 256: 128 pairs of row 16u+g,
    # then 128 pairs of row 16u+8+g; stream[k] at [16g + k%16, 16u + k//16].
    pairs = (dgb[:, 0::2] * NDEP + dgb[:, 1::2]).astype(np.int16)  # [L, 128]
    dgw = np.empty((128, 128), dtype=np.int16)
    for v in range(4):
        for g in range(8):
            stream = np.concatenate([
                pairs[i0 + 16 * ((4 * v + b) // 2) + 8 * ((4 * v + b) % 2) + g]
                for b in range(4)
            ])  # [512]
            dgw[16 * g:16 * (g + 1), 32 * v:32 * (v + 1)] = stream.reshape(32, 16).T
    m = {
        "xbT": np.ascontiguousarray(input_tensor[b].T),
        "xsT": np.ascontiguousarray(input_tensor[b, i0:i0 + RPC].T),
        "dgw": dgw,
    }
    m.update(consts)
    return m


def kernel(input_tensor, dependency_graph, s_fc_w, s_fc_b, t_fc_w, t_fc_b,
           dep_emb, cls_w, cls_b):
    global _PROGRAM
    from concourse.bass_utils import run_bass_kernel_spmd

    input_tensor = np.asarray(input_tensor, dtype=np.float32)
    dg = np.asarray(dependency_graph)
    out_dtype = np.float32

    cw = np.asarray(cls_w, np.float32)
    wpack = np.concatenate([
        cw[:, 0:H].T, cw[:, H:2 * H].T, cw[:, 2 * H:].T,
        np.asarray(dep_emb, np.float32).T,
        np.asarray(s_fc_b, np.float32).reshape(H, 1),
        np.asarray(t_fc_b, np.float32).reshape(H, 1),
    ], axis=1)
    consts = {
        "sfw": np.ascontiguousarray(np.asarray(s_fc_w, np.float32)),
        "tfw": np.ascontiguousarray(np.asarray(t_fc_w, np.float32)),
        "wpack": np.ascontiguousarray(wpack),
        "clbT": np.asarray(cls_b, np.float32).reshape(C, 1).copy(),
    }
    # repmat[c, 16g+c'] = (c' == c)
    rm = np.zeros((C, 128), dtype=np.float32)
    for g in range(8):
        rm[np.arange(C), 16 * g + np.arange(C)] = 1.0
    consts["repmat"] = rm

    if _PROGRAM is None:
        _PROGRAM = build_program()
    nc = _PROGRAM

    in_maps = [_marshal_core(n, input_tensor, dg, consts) for n in range(NCORES)]
    trace = bool(int(os.environ.get("KERNEL_PROFILE", "0")))
    res = run_bass_kernel_spmd(
        nc, in_maps, core_ids=list(range(NCORES)), trace=trace
    )
    if trace and res.exec_time_ns is not None:
        print(f"HW exec time: {res.exec_time_ns} ns")

    out = np.empty((B, L, C, L), dtype=out_dtype)
    for n in range(NCORES):
        b, half = n // 2, n % 2
        i0 = half * RPC
        # raw flat row = (2u+h)*128 + 16g + c ; i_loc = 16u + 8h + g
        raw = res.results[n]["out"].reshape(2 * NINST, 8, 16, L)  # [block=2u+h, g, c16, j]
        out[b, i0:i0 + RPC] = raw[:, :, :C, :].reshape(2 * NINST * 8, C, L)[_ROWPERM]
    return out


# revision 33
# speedup vs baseline: 1.0041x; 1.0041x over previous
"""Trainium2 Bass kernel for nn_DependencyLinearLayer.

Math (collapsed-H reformulation of the reference):
  out[b,i,c,j] = dep_logits[dg[b,i,j], c] + s_log[b,i,c] + t_log[b,j,c] + cls_b[c]
where
  dep_logits = dep_emb @ w_d.T                  [48, 12]
  s_log      = x @ (w_s @ s_fc_w).T + w_s@s_fc_b  (combined-weight form)
  t_log      = x @ (w_t @ t_fc_w).T + w_t@t_fc_b
  w_s, w_t, w_d = cls_w[:, :H], cls_w[:, H:2H], cls_w[:, 2H:]

Sharding: 8 cores; core n handles batch b = n//2 and i-rows [128*(n%2), 128*(n%2)+128).

The per-element 48-entry table lookup runs on GPSIMD via ap_gather with a
PAIRED table: table2[48*a+b] = (T[a], T[b]) so each gather index produces two
consecutive output j's, halving the Q7 read-command count (the bottleneck).
All floating-point math (projections, table construction, broadcast adds)
runs on-device (PE/ACT/DVE); the host only reshapes/shards inputs.
"""

import os
import sys

import numpy as np

for _p in ("/opt/trn_rl_repo",):
    if _p not in sys.path:
        sys.path.insert(0, _p)

import concourse.bass as bass  # noqa: E402
import concourse.tile as tile  # noqa: E402
from concourse import bacc, mybir  # noqa: E402
from concourse.tile import ScopedClock, add_dep_helper  # noqa: E402

B, L, IN, H, C, NDEP = 4, 256, 768, 256, 12, 48
NCORES = 8
RPC = L // 2  # i-rows per core (128)
NINST = 8    # ap_gather instructions per core (2 i-rows x 8 groups each)

_MAX_TAIL_WAITS = 1


def _patched_drain_and_barrier(self, tick_clock, wait_clock):
    # The walrus build in this image rejects >1 sync-wait on one CTRL
    # instruction; split the kernel-tail drain waits across nops.
    drain_inst = self.nc.sync.drain()
    wait_clock.add_sem_waits(
        drain_inst.ins, ScopedClock({None: tick_clock.global_clock})
    )
    sync_info = drain_inst.ins.sync_info
    if sync_info is not None and len(sync_info.on_wait) > _MAX_TAIL_WAITS:
        waits = list(sync_info.on_wait)
        sync_info.on_wait = waits[:_MAX_TAIL_WAITS]
        rest = waits[_MAX_TAIL_WAITS:]
        while rest:
            chunk, rest = rest[:_MAX_TAIL_WAITS], rest[_MAX_TAIL_WAITS:]
            nop = self.nc.sync.nop(nofuse=True, hint="tail_drain_split").ins
            nop.sync_info = mybir.SyncInfo(on_wait=chunk, on_update=[])
    self.nc.all_engine_barrier()
    assert self.sems is not None
    popped = self.nc._tile_sem_poison_stack.pop()
    assert popped is self._sem_poison
    self.nc.clear_and_free_semaphores(list(self.sems.allocated().values()))
    self.nc.all_engine_barrier()


tile.TileContext._drain_and_barrier = _patched_drain_and_barrier

_PROGRAM = None

# raw block order (block=2u+h, g) -> i_loc = 16u + 8h + g; _ROWPERM[i_loc] = block*8+g
_ROWPERM = np.zeros(RPC, dtype=np.int64)
for _u in range(NINST):
    for _h in range(2):
        for _g in range(8):
            _ROWPERM[16 * _u + 8 * _h + _g] = (2 * _u + _h) * 8 + _g


def build_program():
    f32 = mybir.dt.float32
    nc = bacc.Bacc("TRN2", target_bir_lowering=False, debug=False)

    xbT = nc.declare_dram_parameter("xbT", [IN, L], f32, isOutput=False)
    xsT = nc.declare_dram_parameter("xsT", [IN, RPC], f32, isOutput=False)
    dgw = nc.declare_dram_parameter("dgw", [128, 128], mybir.dt.int16, isOutput=False)
    sfw = nc.declare_dram_parameter("sfw", [H, IN], f32, isOutput=False)
    tfw = nc.declare_dram_parameter("tfw", [H, IN], f32, isOutput=False)
    # packed small weights: cols [wsT|wtT|wdT|depT|sfb|tfb] = 12+12+12+48+1+1
    wpack = nc.declare_dram_parameter("wpack", [H, 86], f32, isOutput=False)
    clbT = nc.declare_dram_parameter("clbT", [C, 1], f32, isOutput=False)
    repmat = nc.declare_dram_parameter("repmat", [C, 128], f32, isOutput=False)
    out_d = nc.declare_dram_parameter("out", [RPC * 16, L], f32, isOutput=True)

    Copy = mybir.ActivationFunctionType.Copy

    with tile.TileContext(nc) as tc:
        with (
            tc.tile_pool(name="const", bufs=1) as cp,
            tc.tile_pool(name="gpool", bufs=8) as gp,
            tc.tile_pool(name="psum", bufs=1, space="PSUM") as pp,
            tc.tile_pool(name="psum2", bufs=1, space="PSUM") as pp2,
        ):
            # ---- input loads: small gather-critical first (sync queue
            # spreads across DMA engines; scalar queue serializes ~1.3us/desc) ----
            warm_tab = cp.tile([128, 4], f32, tag="warm_tab")
            warm_out = cp.tile([128, 16], f32, tag="warm_out")
            wpk = cp.tile([128, 172], f32, tag="wpk")
            nc.sync.dma_start(
                wpk[:].rearrange("p (h q) -> p h q", h=2),
                wpack[:].rearrange("(h p) q -> p h q", h=2),
            )

            def wp(h0, a, b):
                return wpk[:, h0 * 86 + a: h0 * 86 + b]
            clbT_t = cp.tile([C, 1], f32, tag="clbT_t")
            nc.sync.dma_start(clbT_t[:], clbT[:])
            rep_t = cp.tile([C, 128], f32, tag="rep_t")
            nc.sync.dma_start(rep_t[:], repmat[:])
            dgw_t = cp.tile([128, 128], mybir.dt.int16, tag="dgw_t")
            nc.sync.dma_start(dgw_t[:], dgw[:])
            with tc.high_priority():
                warm_src = wpk[:, 24:28]
                nc.vector.tensor_scalar_mul(warm_tab[:], warm_src, 0.0)
                nc.gpsimd.ap_gather(
                    warm_out[:], warm_tab[:],
                    warm_tab[:, 0:1].bitcast(mybir.dt.int16)[:, 0:1],
                    channels=128, num_elems=4, d=1, num_idxs=16,
                )
            # big loads after the critical batch
            sfw_t = cp.tile([128, 2 * IN], f32, tag="sfw_t")
            tfw_t = cp.tile([128, 2 * IN], f32, tag="tfw_t")
            for h0 in range(2):
                nc.sync.dma_start(sfw_t[:, h0 * IN:(h0 + 1) * IN], sfw[h0 * 128:(h0 + 1) * 128, :])
                nc.sync.dma_start(tfw_t[:, h0 * IN:(h0 + 1) * IN], tfw[h0 * 128:(h0 + 1) * 128, :])
            x_t = cp.tile([128, 6 * L], f32, tag="x_t")
            for m in range(6):
                nc.sync.dma_start(x_t[:, m * L:(m + 1) * L], xbT[m * 128:(m + 1) * 128, :])
            xs_t = cp.tile([128, 6 * RPC], f32, tag="xs_t")
            for m in range(6):
                nc.sync.dma_start(xs_t[:, m * RPC:(m + 1) * RPC], xsT[m * 128:(m + 1) * 128, :])

            # ---- dep_logitsT+biases [12, 48] in ONE accumulation group ----
            # pd[c,d] = sum_h wd[h,c]*depT[h,d] + ws@sfb + wt@tfb (bcast over d)
            with tc.high_priority():
                pd = pp.tile([C, NDEP], f32, tag="pd")
                nc.tensor.matmul(pd[:], wp(0, 24, 36), wp(0, 36, 84),
                                 start=True, stop=False)
                nc.tensor.matmul(pd[:], wp(1, 24, 36), wp(1, 36, 84),
                                 start=False, stop=False)
                nc.tensor.matmul(pd[:], wp(0, 0, 12),
                                 wp(0, 84, 85).broadcast_to([128, NDEP]),
                                 start=False, stop=False)
                nc.tensor.matmul(pd[:], wp(1, 0, 12),
                                 wp(1, 84, 85).broadcast_to([128, NDEP]),
                                 start=False, stop=False)
                nc.tensor.matmul(pd[:], wp(0, 12, 24),
                                 wp(0, 85, 86).broadcast_to([128, NDEP]),
                                 start=False, stop=False)
                pd_last = nc.tensor.matmul(pd[:], wp(1, 12, 24),
                                 wp(1, 85, 86).broadcast_to([128, NDEP]),
                                 start=False, stop=True)

            with tc.high_priority():
                midbufD = cp.tile([C, NDEP], f32, tag="midbufD")
                nc.scalar.activation(midbufD[:], pd[:],
                                     mybir.ActivationFunctionType.Identity, bias=clbT_t[:])
                prepD = pp2.tile([128, NDEP], f32, tag="prepD")
                prep_mm = nc.tensor.matmul(prepD[:], rep_t[:], midbufD[:],
                                           start=True, stop=True)
                t16_t = cp.tile([128, NDEP], f32, tag="t16_t")
                nc.vector.tensor_copy(t16_t[:], prepD[:])

                table2 = cp.tile([128, NDEP * NDEP * 2], f32, tag="table2")
                tv = table2[:].rearrange("p (a b t) -> p a b t", a=NDEP, b=NDEP, t=2)
                nc.vector.tensor_copy(
                    tv[:, :, :, 0], t16_t[:].unsqueeze(2).broadcast_to([128, NDEP, NDEP])
                )
                copy_a = nc.vector.tensor_copy(
                    tv[:, 0:12, :, 1],
                    t16_t[:].unsqueeze(1).broadcast_to([128, 12, NDEP]),
                )
                copy_b = nc.scalar.copy(
                    tv[:, 12:NDEP, :, 1],
                    t16_t[:].unsqueeze(1).broadcast_to([128, NDEP - 12, NDEP]),
                )

            # ---- combined weights W2[k, 0:12]=swT, [12:24]=twT ----
            w2_t = cp.tile([128, 6 * 24], f32, tag="w2_t")
            for m in range(6):
                pw = pp.tile([128, 24], f32, tag="pw")
                for h0 in range(2):
                    mm = nc.tensor.matmul(
                        pw[:, 0:C],
                        sfw_t[:, h0 * IN + m * 128: h0 * IN + (m + 1) * 128],
                        wp(h0, 0, 12),
                        start=(h0 == 0), stop=(h0 == 1),
                    )
                    if m == 0 and h0 == 0:
                        add_dep_helper(mm.ins, pd_last.ins, sync=False,
                                       reason="pd group first on PE")
                        add_dep_helper(mm.ins, prep_mm.ins, sync=False,
                                       reason="prepD before combine on PE")
                for h0 in range(2):
                    nc.tensor.matmul(
                        pw[:, C:2 * C],
                        tfw_t[:, h0 * IN + m * 128: h0 * IN + (m + 1) * 128],
                        wp(h0, 12, 24),
                        start=(h0 == 0), stop=(h0 == 1),
                    )
                w2c = nc.vector.tensor_copy(w2_t[:, m * 24:(m + 1) * 24], pw[:])
                if m == 0:
                    add_dep_helper(w2c.ins, copy_a.ins, sync=False,
                                   reason="table copyA first on DVE")


            # ---- projections: s_logT [12, 256] and t_logT [12, 256] ----
            ps = pp2.tile([C, RPC], f32, tag="ps")
            pt = pp2.tile([C, L], f32, tag="pt")
            for m in range(6):
                nc.tensor.matmul(
                    ps[:], w2_t[:, m * 24: m * 24 + C], xs_t[:, m * RPC:(m + 1) * RPC],
                    start=(m == 0), stop=(m == 5),
                )
            for m in range(6):
                nc.tensor.matmul(
                    pt[:], w2_t[:, m * 24 + C: m * 24 + 2 * C], x_t[:, m * L:(m + 1) * L],
                    start=(m == 0), stop=(m == 5),
                )



            # ---- t/s projections epilogue ----
            midbufT = cp.tile([C, L], f32, tag="midbufT")
            mbt = nc.scalar.activation(midbufT[:], pt[:], Copy)
            add_dep_helper(mbt.ins, copy_b.ins, sync=False,
                           reason="table copyB first on ACT")
            slog_t = cp.tile([C, RPC], f32, tag="slog_t")
            nc.scalar.activation(slog_t[:], ps[:], Copy)
            prepT = pp2.tile([128, L], f32, tag="prepT")
            nc.tensor.matmul(prepT[:], rep_t[:], midbufT[:], start=True, stop=True)
            t16u16 = cp.tile([128, L], f32, tag="t16u16")
            nc.vector.tensor_copy(t16u16[:], prepT[:])

            # ---- S_all [128, 16]: per-(instruction, half) per-partition scalars ----
            # S_all[16g+c, 2u+h] = s_logT[c, 16u+8h+g]
            s_all = cp.tile([128, 16], f32, tag="s_all")
            nc.vector.tensor_scalar_mul(s_all[:], sfw_t[:, 0:16], 0.0)
            for g in range(8):
                nc.scalar.dma_start(
                    s_all[16 * g:16 * g + C, :],
                    slog_t[0:C, g:g + 121:8],
                )

            # ---- gathers + fused adds + stores ----
            u16 = t16u16[:]
            g_tiles = []
            gather_insts = []
            for v in range(3):
                g_t = gp.tile([128, 4 * L], f32, tag="g_t")
                gi = nc.gpsimd.ap_gather(
                    g_t[:].rearrange("p (k t) -> p k t", t=2),
                    table2[:].rearrange("p (e t) -> p e t", t=2),
                    dgw_t[:, 32 * v:32 * (v + 1)],
                    channels=128, num_elems=NDEP * NDEP, d=2, num_idxs=512,
                )
                g_tiles.append(g_t)
                gather_insts.append(gi)
            # last block-pair split in two so its first half's epilogue
            # overlaps the second half's gather
            g_t3 = gp.tile([128, 4 * L], f32, tag="g_t3")
            for half in range(2):
                gi = nc.gpsimd.ap_gather(
                    g_t3[:, half * 2 * L:(half + 1) * 2 * L].rearrange(
                        "p (k t) -> p k t", t=2),
                    table2[:].rearrange("p (e t) -> p e t", t=2),
                    dgw_t[:, 96 + 16 * half:96 + 16 * (half + 1)],
                    channels=128, num_elems=NDEP * NDEP, d=2, num_idxs=256,
                )
                gather_insts.append(gi)
            g_tiles.append(g_t3)
            # adds + stores AFTER all gathers (DVE shares the GPSIMD SBUF port)
            last_g = gather_insts[3]  # first half of the split pair
            for v in range(4):
                g_t = g_tiles[v]
                for bidx in range(4):
                    aa = nc.vector.affine_then_add(
                        g_t[:, bidx * L:(bidx + 1) * L],
                        g_t[:, bidx * L:(bidx + 1) * L],
                        u16,
                        1.0,
                        s_all[:, 4 * v + bidx:4 * v + bidx + 1],
                    )
                    if v < 3:
                        add_dep_helper(aa.ins, last_g.ins, sync=False,
                                       reason="defer adds past gathers")
                if v < 3:
                    nc.sync.dma_start(
                        out_d[4 * v * 128:(4 * v + 4) * 128, :].rearrange(
                            "(b p) j -> p b j", b=4
                        ),
                        g_t[:].rearrange("p (b j) -> p b j", b=4),
                    )
                else:
                    for half in range(2):
                        nc.sync.dma_start(
                            out_d[(12 + 2 * half) * 128:(14 + 2 * half) * 128, :]
                            .rearrange("(b p) j -> p b j", b=2),
                            g_t[:, half * 2 * L:(half + 1) * 2 * L]
                            .rearrange("p (b j) -> p b j", b=2),
                        )

    nc.compile()
    return nc


def _marshal_core(n, input_tensor, dg, consts):
    b, half = n // 2, n % 2
    i0 = half * RPC
    dgb = dg[b]
    # paired indices, wrapped per 16-partition group:
    # instruction u, group g -> stream of 256: 128 pairs of row 16u+g,
    # then 128 pairs of row 16u+8+g; stream[k] at [16g + k%16, 16u + k//16].
    pairs = (dgb[:, 0::2] * NDEP + dgb[:, 1::2]).astype(np.int16)  # [L, 128]
    dgw = np.empty((128, 128), dtype=np.int16)
    for v in range(4):
        for g in range(8):
            stream = np.concatenate([
                pairs[i0 + 16 * ((4 * v + b) // 2) + 8 * ((4 * v + b) % 2) + g]
                for b in range(4)
            ])  # [512]
            dgw[16 * g:16 * (g + 1), 32 * v:32 * (v + 1)] = stream.reshape(32, 16).T
    m = {
        "xbT": np.ascontiguousarray(input_tensor[b].T),
        "xsT": np.ascontiguousarray(input_tensor[b, i0:i0 + RPC].T),
        "dgw": dgw,
    }
    m.update(consts)
    return m


def kernel(input_tensor, dependency_graph, s_fc_w, s_fc_b, t_fc_w, t_fc_b,
           dep_emb, cls_w, cls_b):
    global _PROGRAM
    from concourse.bass_utils import run_bass_kernel_spmd

    input_tensor = np.asarray(input_tensor, dtype=np.float32)
    dg = np.asarray(dependency_graph)
    out_dtype = np.float32

    cw = np.asarray(cls_w, np.float32)
    wpack = np.concatenate([
        cw[:, 0:H].T, cw[:, H:2 * H].T, cw[:, 2 * H:].T,
        np.asarray(dep_emb, np.float32).T,
        np.asarray(s_fc_b, np.float32).reshape(H, 1),
        np.asarray(t_fc_b, np.float32).reshape(H, 1),
    ], axis=1)
    consts = {
        "sfw": np.ascontiguousarray(np.asarray(s_fc_w, np.float32)),
        "tfw": np.ascontiguousarray(np.asarray(t_fc_w, np.float32)),
        "wpack": np.ascontiguousarray(wpack),
        "clbT": np.asarray(cls_b, np.float32).reshape(C, 1).copy(),
    }
    # repmat[c, 16g+c'] = (c' == c)
    rm = np.zeros((C, 128), dtype=np.float32)
    for g in range(8):
        rm[np.arange(C), 16 * g + np.arange(C)] = 1.0
    consts["repmat"] = rm

    if _PROGRAM is None:
        _PROGRAM = build_program()
    nc = _PROGRAM

    in_maps = [_marshal_core(n, input_tensor, dg, consts) for n in range(NCORES)]
    trace = bool(int(os.environ.get("KERNEL_PROFILE", "0")))
    res = run_bass_kernel_spmd(
        nc, in_maps, core_ids=list(range(NCORES)), trace=trace
    )
    if trace and res.exec_time_ns is not None:
        print(f"HW exec time: {res.exec_time_ns} ns")

    out = np.empty((B, L, C, L), dtype=out_dtype)
    for n in range(NCORES):
        b, half = n // 2, n % 2
        i0 = half * RPC
        # raw flat row = (2u+h)*128 + 16g + c ; i_loc = 16u + 8h + g
        raw = res.results[n]["out"].reshape(2 * NINST, 8, 16, L)  # [block=2u+h, g, c16, j]
        out[b, i0:i0 + RPC] = raw[:, :, :C, :].reshape(2 * NINST * 8, C, L)[_ROWPERM]
    return out
